# revision 30
# baseline (speedup 1.0000x reference)
"""Differential attention (DIFF Transformer layer) on 8 Trainium2 NeuronCores.

Sharding: tensor-parallel over heads x data-parallel over batch.
Core c (0..7) handles batch b = c//4 and the head-quad qd = c%4
(heads 4*qd .. 4*qd+3 of 16, BOTH score groups). Each core computes its
heads' q/k/v projections, causal softmax attention for both groups,
the differential combine (a1@v1 - lam*a2@v2)*(1-lam_init), and a
row-parallel partial of the output projection. The host sums the 4
partial outputs per batch (the unshard step of row-parallel TP).

Kernel structure per core (all matmuls fp16, PSUM fp32):
  0. HAM warm-up matmuls flip the PE clock gate to 8/8 while the first
     x DMA is in flight
  1. x_b -> x^T in SBUF via regular matmuls against a moving identity
     (2x faster than transpose-mode, counts as PE-busy for HAM)
  2. q^T, k^T = W^T @ x^T (dense per-head layouts), v natural
  3. flash-style causal attention per (head): the two score groups have
     true contraction 64, so their score matmuls run CONCURRENTLY in
     different PE row-strips (tile_position row tiling); one batched
     EXP per k-chunk covers both groups (halves ACT instruction
     overhead); diagonal blocks masked with affine_select; o[q,:]
     accumulated in PSUM via A^T-chunk-stationary matmuls against
     V'=[V|1] (the ones column yields softmax row sums for free)
  4. normalize by row sums, combine groups, transpose o, o @ Wo slice
"""

import numpy as np

import concourse.bass as bass
import concourse.mybir as mybir
import concourse.tile as tile
from concourse.bass_utils import run_bass_kernel_spmd
from concourse.masks import make_identity
from contextlib import ExitStack


_MAX_WAITS = 1  # walrus setupSyncWait caps sem-waits per instruction


def _spill_excess_waits(nc):
    """This walrus build rejects instructions carrying more than a couple
    of sem-waits (setupSyncWait: 'Too many sync wait commands'). Move the
    excess onto same-engine NoOps inserted just before the instruction —
    the engine blocks on the NoOps' waits first, so semantics match."""
    idx = 0
    for f in nc.m.functions:
        for bb in f.blocks:
            new = []
            changed = False
            for inst in bb.instructions:
                si = getattr(inst, "sync_info", None)
                waits = list(si.on_wait) if si is not None and si.on_wait else []
                if (
                    len(waits) > _MAX_WAITS
                    and inst.engine != mybir.EngineType.Unassigned
                ):
                    changed = True
                    excess = waits[: -_MAX_WAITS]
                    for j in range(0, len(excess), _MAX_WAITS):
                        nop = mybir.InstNoOp(
                            name=f"wspill-{idx}",
                            bass_nofuse=True,
                            sync_info=mybir.SyncInfo(
                                on_wait=excess[j : j + _MAX_WAITS], on_update=[]
                            ),
                        )
                        idx += 1
                        nop.engine = inst.engine
                        nc.register_instruction(nop)
                        new.append(nop)
                    si.on_wait = waits[-_MAX_WAITS:]
                new.append(inst)
            if changed:
                bb.instructions = new


_orig_drain_and_barrier = tile.TileContext._drain_and_barrier


def _drain_barrier_and_spill(self, tick_clock, wait_clock):
    _orig_drain_and_barrier(self, tick_clock, wait_clock)
    _spill_excess_waits(self.nc)


tile.TileContext._drain_and_barrier = _drain_barrier_and_spill

P = 128
S = 2048
D = 1024
DH = 64
NH_TOT = 16
NHC = 4  # heads per core
NG = 2  # score groups
LAMBDA_INIT = 0.8
NCORES = 8

F32 = mybir.dt.float32
F32R = mybir.dt.float32r
BF16 = mybir.dt.bfloat16
F16 = mybir.dt.float16
EXP = mybir.ActivationFunctionType.Exp
MULT = mybir.AluOpType.mult
IS_GE = mybir.AluOpType.is_ge

TOKC = S // P  # 16 token chunks
DC = D // P  # 8 d_model chunks
QB = 512  # q block width for score matmuls
NQB = S // QB  # 4
WCOLS = NHC * NG * DH  # 512 projection cols per core
OROWS = NHC * DH  # 256 o_proj rows per core

LAST_RESULT = None  # test harness reads exec_time_ns from here


def _r(ap):
    return ap.bitcast(F32R)


def build_program(c1: float, c2: float) -> bass.Bass:
    """c1 = (1-lambda_init), c2 = (1-lambda_init)*lambda — baked immediates."""
    nc = bass.Bass("TRN2", target_bir_lowering=False, debug=False)

    xb = nc.dram_tensor("xb", [S, D], F32, kind="ExternalInput").ap()
    wq = nc.dram_tensor("wq", [D, WCOLS], F32, kind="ExternalInput").ap()
    wk = nc.dram_tensor("wk", [D, WCOLS], F32, kind="ExternalInput").ap()
    wv = nc.dram_tensor("wv", [D, WCOLS], F32, kind="ExternalInput").ap()
    wo = nc.dram_tensor("wo", [OROWS, D], F32, kind="ExternalInput").ap()
    out = nc.dram_tensor("out", [S, D], F16, kind="ExternalOutput").ap()

    NQ = 4  # token quarters (== q blocks)

    with tile.TileContext(nc) as tc, ExitStack() as es:
        pool = es.enter_context(tc.tile_pool(name="main", bufs=1))
        ident16 = pool.tile([P, P], F16)
        make_identity(nc, ident16)

        # Tensors are split per token quarter so the Tile scheduler can
        # overlap attention on early quarters with projections of later ones.
        qTq = [pool.tile([P, NHC, QB], F16, name=f"qT{j}") for j in range(NQ)]
        # k^T strips: per (head, group), the group's 64 rows at their natural
        # position, zeros in the other half — a full-128 stationary masks the
        # other group in the stacked q^T moving operand (K=128 keeps FWL on;
        # K=64 row-tiled pairs measured slower)
        kTq = [pool.tile([P, NHC * NG, QB], F16, name=f"kT{j}") for j in range(NQ)]
        vSq = [
            pool.tile([P, 4, NHC * NG, DH + 1], F16, name=f"vS{j}") for j in range(NQ)
        ]
        xTq = [pool.tile([P, DC, QB], F16, name=f"xT{j}") for j in range(NQ)]

        xs_pool = es.enter_context(tc.tile_pool(name="xs", bufs=5))
        xc_pool = es.enter_context(tc.tile_pool(name="xc", bufs=4))
        a_pool = es.enter_context(tc.tile_pool(name="a", bufs=6))
        nrm_pool = es.enter_context(tc.tile_pool(name="nrm", bufs=4))
        od_pool = es.enter_context(tc.tile_pool(name="odq", bufs=2))
        odT_pool = es.enter_context(tc.tile_pool(name="odT", bufs=2))
        outs_pool = es.enter_context(tc.tile_pool(name="outs", bufs=4))
        # PSUM (8 banks): 2 proj/o_proj/transpose (tag ps), 2x2-bank score
        # tiles (both groups side by side), 2 AV accumulators
        pp512 = es.enter_context(tc.tile_pool(name="pp512", bufs=2, space="PSUM"))
        s_psum = es.enter_context(tc.tile_pool(name="sps", bufs=2, space="PSUM"))
        o_psum = es.enter_context(tc.tile_pool(name="ops", bufs=2, space="PSUM"))

        # HAM warm-up: ~36 back-to-back dummy matmuls keep the PE busy for
        # >3.4us starting at t~8.5us, flipping the clock gate to 8/8 before
        # the real transposes/projections arrive (they'd otherwise run at
        # 1.2GHz until ~40us in).
        warm = pp512.tile([P, P], F32, tag="ps", name="warm")
        for _ in range(24):
            nc.tensor.matmul(
                warm[:], lhsT=ident16[:], rhs=ident16[:], start=True, stop=True
            )

        # ---- projections, one token quarter at a time ----
        for j in range(NQ):
            for ti in range(4):
                t = j * 4 + ti
                xstage = xs_pool.tile([P, D], F32, tag="xs", name="xs")
                nc.sync.dma_start(xstage[:], xb[t * P : (t + 1) * P, :])
                xc = xc_pool.tile([P, D], F16, tag="xc", name="xc")
                if ti % 2:
                    nc.scalar.copy(xc[:], xstage[:])
                else:
                    nc.vector.tensor_copy(xc[:], xstage[:])
                for dq in range(DC // 4):
                    # transpose as regular matmuls (stationary = chunk,
                    # moving = identity): ~2x faster than transpose-mode.
                    # 4 transposes land in one psum bank (disjoint regions,
                    # start only on the first clears has_written) so a single
                    # wide copy drains them.
                    pt4 = o_psum.tile([P, 4, P], F32, tag="og", name="tp")
                    for c in range(4):
                        dc = dq * 4 + c
                        nc.tensor.matmul(
                            pt4[:, c, :],
                            lhsT=xc[:, dc * P : (dc + 1) * P],
                            rhs=ident16[:],
                            start=(c == 0),
                            stop=(c == 3),
                        )
                    nc.vector.tensor_copy(
                        xTq[j][:, dq * 4 : dq * 4 + 4, ti * P : (ti + 1) * P],
                        pt4[:],
                    )
            if j == 0:
                # weights + constants load after quarter-0's x pipeline is
                # queued, so the PE can start transposing immediately
                wst_pool = es.enter_context(tc.tile_pool(name="wst", bufs=5))
                w16 = {}
                nw = 0
                for nm, wdram in (("q", wq), ("k", wk), ("v", wv)):
                    w16[nm] = [
                        pool.tile([P, WCOLS], F16, name=f"w16{nm}{dc}") for dc in range(DC)
                    ]
                    for dc in range(DC):
                        wst = wst_pool.tile([P, WCOLS], F32, tag="wst", name="wst")
                        nc.sync.dma_start(wst[:], wdram[dc * P : (dc + 1) * P, :])
                        # alternate engines so neither serializes the PE feed
                        eng = nc.scalar.copy if nw % 2 else nc.vector.tensor_copy
                        eng(w16[nm][dc][:], wst[:])
                        nw += 1
                wos = pool.tile([P, OROWS // P, D], F16)
            if j == 1:
                # wo is not needed until the first o_proj (~t=75us); load it
                # after quarter-1's x so it delays neither
                for mc in range(OROWS // P):
                    wst = wst_pool.tile([P, D], F32, tag="wst", name="wost")
                    nc.sync.dma_start(wst[:], wo[mc * P : (mc + 1) * P, :])
                    nc.scalar.copy(wos[:, mc, :], wst[:])
                for jj in range(NQ):
                    for g in range(NG):
                        other = (1 - g) * DH
                        for hh in range(NHC):
                            nc.gpsimd.memset(
                                kTq[jj][other : other + DH, 2 * hh + g, :], 0.0
                            )
                    nc.gpsimd.memset(vSq[jj][:, :, :, DH], 1.0)

            # q^T, k^T: out[dims 128, tok 512]; one live psum per dim chunk.
            # PSUM->SBUF copies go on the Scalar engine, idle in this phase
            # (the DVE is busy with x casts + transpose copies).
            for nm in ("q", "k"):
                for mc in range(NHC):
                    ps = pp512.tile([P, QB], F32, tag="ps", name="ps")
                    for dc in range(DC):
                        nc.tensor.matmul(
                            ps[:],
                            lhsT=w16[nm][dc][:, mc * P : (mc + 1) * P],
                            rhs=xTq[j][:, dc, :],
                            start=(dc == 0),
                            stop=(dc == DC - 1),
                        )
                    if nm == "q":
                        eng = nc.scalar.copy if mc % 2 else nc.vector.tensor_copy
                        eng(qTq[j][:, mc, :], ps[:])
                    else:
                        eng = nc.scalar.copy if mc % 2 else nc.vector.tensor_copy
                        for g in range(NG):
                            eng(
                                kTq[j][g * DH : (g + 1) * DH, 2 * mc + g, :],
                                ps[g * DH : (g + 1) * DH, :],
                            )
            # v: out[tok 128, strips 512]
            for ti in range(4):
                ps = pp512.tile([P, QB], F32, tag="ps", name="ps")
                for dc in range(DC):
                    nc.tensor.matmul(
                        ps[:],
                        lhsT=xTq[j][:, dc, ti * P : (ti + 1) * P],
                        rhs=w16["v"][dc][:],
                        start=(dc == 0),
                        stop=(dc == DC - 1),
                    )
                eng = nc.scalar.copy if ti % 2 else nc.vector.tensor_copy
                eng(
                    vSq[j][:, ti, :, 0:DH],
                    ps[:].rearrange("p (s d) -> p s d", s=NHC * NG),
                )

        # ---- attention + per-q-block o_proj ----
        for qb in range(NQB):
            o_dq = od_pool.tile([P, 4, OROWS], F16, tag="odq", name="odq")

            def do_av(hh, og, kc, at):
                kj, ki = kc // 4, kc % 4
                for g in range(NG):
                    strip = 2 * hh + g
                    for qs in range(4):
                        if kc - 4 * qb > qs:
                            continue  # fully masked sub-block
                        # one accumulation group per og bank: the first
                        # matmul's start clears has_written for the whole
                        # bank; later matmuls overwrite where unwritten /
                        # accumulate where written
                        nc.tensor.matmul(
                            og[g][:, qs, :],
                            lhsT=at[:, g, qs * P : (qs + 1) * P],
                            rhs=vSq[kj][:, ki, strip, :],
                            start=(kc == 0 and qs == 0),
                            stop=(kc == 4 * qb + 3 and qs == 3),
                        )

            def emit_norm(hh, og):
                # normalize rows, combine groups: o = c1*o1/s1 - c2*o2/s2
                rc = [
                    nrm_pool.tile([P, 4, 1], F32, tag="rc", name="rc")
                    for _ in range(NG)
                ]
                for g in range(NG):
                    nc.vector.reciprocal(rc[g][:], og[g][:, :, DH : DH + 1])
                    nc.vector.tensor_scalar_mul(
                        rc[g][:], rc[g][:], c1 if g == 0 else -c2
                    )
                t0 = nrm_pool.tile([P, 4, DH], F32, tag="tt")
                t1 = nrm_pool.tile([P, 4, DH], F32, tag="tt")
                nc.vector.tensor_tensor(
                    t0[:], og[0][:, :, 0:DH], rc[0][:].to_broadcast([P, 4, DH]), MULT
                )
                nc.vector.tensor_tensor(
                    t1[:], og[1][:, :, 0:DH], rc[1][:].to_broadcast([P, 4, DH]), MULT
                )
                nc.vector.tensor_add(
                    o_dq[:, :, hh * DH : (hh + 1) * DH], t0[:], t1[:]
                )

            # software pipeline ACROSS head chains: AV of chunk kc is emitted
            # two score/exp groups later (possibly inside the next head's
            # chain), so AV stationary loads prefetch under long score
            # matmuls and a chain's final exp lag is absorbed by the next
            # chain's scores instead of stalling the PE FIFO
            kmax = 4 * (qb + 1) - 1
            pendq = []

            def pop_av():
                hh_, og_, kc_, at_ = pendq.pop(0)
                do_av(hh_, og_, kc_, at_)
                if kc_ == kmax:
                    emit_norm(hh_, og_)

            for hh in range(NHC):
                og = [
                    o_psum.tile([P, 4, DH + 1], F32, tag="og", name="og")
                    for _ in range(NG)
                ]
                for kc in range(4 * (qb + 1)):
                    # drain a deferred AV group BEFORE this chunk's score
                    # matmuls: the PE then fills the sp-ring wait (exp of
                    # kc-1 freeing its slot) with AV work instead of idling
                    if len(pendq) >= 2:
                        pop_av()
                    kj, ki = kc // 4, kc % 4
                    r = max(0, (kc - 4 * qb) * P)
                    sp = s_psum.tile([P, NG, QB], F32, tag="sp", name="sp")
                    at = a_pool.tile([P, NG, QB], F16, tag="at", name="at")
                    # stacked-group trick: the zero-padded kT strip masks the
                    # other group, so K=128 (full FWL rate) per group
                    for g in range(NG):
                        nc.tensor.matmul(
                            sp[:, g, r:QB],
                            lhsT=kTq[kj][:, 2 * hh + g, ki * P : (ki + 1) * P],
                            rhs=qTq[qb][:, hh, r:QB],
                            start=True,
                            stop=True,
                        )
                    # one EXP for both groups (no max-subtraction: |s|<3)
                    nc.scalar.activation(
                        at[:, :, r:QB], sp[:, :, r:QB], EXP, scale=0.125
                    )
                    if kc >= 4 * qb:
                        # band [r, r+128): keep where col >= row
                        for g in range(NG):
                            nc.gpsimd.affine_select(
                                out=at[:, g, r : r + P],
                                in_=at[:, g, r : r + P],
                                compare_op=IS_GE,
                                fill=0.0,
                                base=0,
                                pattern=[[1, P]],
                                channel_multiplier=-1,
                            )
                    pendq.append((hh, og, kc, at))
            while pendq:
                pop_av()
            # o_proj for this q block, hidden under later attention
            odT = odT_pool.tile([P, OROWS // P, 4 * P], F16, tag="odT", name="odT")
            for mc in range(OROWS // P):
                pt4 = o_psum.tile([P, 4, P], F32, tag="og", name="tp2")
                for tix in range(4):
                    nc.tensor.matmul(
                        pt4[:, tix, :],
                        lhsT=o_dq[:, tix, mc * P : (mc + 1) * P],
                        rhs=ident16[:],
                        start=(tix == 0),
                        stop=(tix == 3),
                    )
                nc.vector.tensor_copy(
                    odT[:, mc, :],
                    pt4[:].rearrange("p c w -> p (c w)"),
                )
            for tix in range(4):
                t = qb * 4 + tix
                for nb in range(D // QB):
                    op = pp512.tile([P, QB], F32, tag="ps", name="op")
                    for mc in range(OROWS // P):
                        nc.tensor.matmul(
                            op[:],
                            lhsT=odT[:, mc, tix * P : (tix + 1) * P],
                            rhs=wos[:, mc, nb * QB : (nb + 1) * QB],
                            start=(mc == 0),
                            stop=(mc == OROWS // P - 1),
                        )
                    ot = outs_pool.tile([P, QB], F16, tag="ot", name="ot")
                    nc.vector.tensor_copy(ot[:], op[:])
                    nc.sync.dma_start(
                        out[t * P : (t + 1) * P, nb * QB : (nb + 1) * QB], ot[:]
                    )

    return nc


_PROGRAM_CACHE: dict = {}


def _get_program(c1: float, c2: float) -> bass.Bass:
    key = (round(c1, 12), round(c2, 12))
    if key not in _PROGRAM_CACHE:
        _PROGRAM_CACHE[key] = build_program(c1, c2)
    return _PROGRAM_CACHE[key]


def make_in_maps(x, Wq, Wk, Wv, Wo):
    """Shard full inputs into the 8 per-core input dicts."""
    x = np.asarray(x, np.float32)
    in_maps = []
    for c in range(NCORES):
        b, qd = divmod(c, 4)
        cols = np.concatenate(
            [
                np.arange(DH) + g * (NH_TOT * DH) + (4 * qd + hh) * DH
                for hh in range(NHC)
                for g in range(NG)
            ]
        )
        in_maps.append(
            {
                "xb": np.ascontiguousarray(x[b]),
                "wq": np.ascontiguousarray(np.asarray(Wq, np.float32)[:, cols]),
                "wk": np.ascontiguousarray(np.asarray(Wk, np.float32)[:, cols]),
                "wv": np.ascontiguousarray(np.asarray(Wv, np.float32)[:, cols]),
                "wo": np.ascontiguousarray(
                    np.asarray(Wo, np.float32)[qd * OROWS : (qd + 1) * OROWS, :]
                ),
            }
        )
    return in_maps


def kernel(x, Wq, Wk, Wv, Wo, lq1, lk1, lq2, lk2):
    global LAST_RESULT
    lam = float(
        np.exp(np.float32(np.dot(lq1, lk1)))
        - np.exp(np.float32(np.dot(lq2, lk2)))
        + np.float32(LAMBDA_INIT)
    )
    c1 = 1.0 - LAMBDA_INIT
    c2 = (1.0 - LAMBDA_INIT) * lam
    nc = _get_program(c1, c2)
    in_maps = make_in_maps(x, Wq, Wk, Wv, Wo)
    res = run_bass_kernel_spmd(nc, in_maps, list(range(NCORES)))
    LAST_RESULT = res
    B = 2
    out64 = np.zeros((B, S, D), np.float64)
    for c in range(NCORES):
        out64[c // 4] += res.results[c]["out"].astype(np.float64)
    return out64.astype(np.float32)


# revision 31
# speedup vs baseline: 1.0407x; 1.0407x over previous
"""Differential attention (DIFF Transformer layer) on 8 Trainium2 NeuronCores.

Sharding: tensor-parallel over heads x data-parallel over batch.
Core c (0..7) handles batch b = c//4 and the head-quad qd = c%4
(heads 4*qd .. 4*qd+3 of 16, BOTH score groups). Each core computes its
heads' q/k/v projections, causal softmax attention for both groups,
the differential combine (a1@v1 - lam*a2@v2)*(1-lam_init), and a
row-parallel partial of the output projection. The host sums the 4
partial outputs per batch (the unshard step of row-parallel TP).

Kernel structure per core (all matmuls fp16, PSUM fp32):
  0. HAM warm-up matmuls flip the PE clock gate to 8/8 while the first
     x DMA is in flight
  1. x_b -> x^T in SBUF via regular matmuls against a moving identity
     (2x faster than transpose-mode, counts as PE-busy for HAM)
  2. q^T, k^T = W^T @ x^T (dense per-head layouts), v natural
  3. flash-style causal attention per (head): the two score groups have
     true contraction 64, so their score matmuls run CONCURRENTLY in
     different PE row-strips (tile_position row tiling); one batched
     EXP per k-chunk covers both groups (halves ACT instruction
     overhead); diagonal blocks masked with affine_select; o[q,:]
     accumulated in PSUM via A^T-chunk-stationary matmuls against
     V'=[V|1] (the ones column yields softmax row sums for free)
  4. normalize by row sums, combine groups, transpose o, o @ Wo slice
"""

import numpy as np

import concourse.bass as bass
import concourse.mybir as mybir
import concourse.tile as tile
from concourse.bass_utils import run_bass_kernel_spmd
from concourse.masks import make_identity
from contextlib import ExitStack


_MAX_WAITS = 1  # walrus setupSyncWait caps sem-waits per instruction


def _spill_excess_waits(nc):
    """This walrus build rejects instructions carrying more than a couple
    of sem-waits (setupSyncWait: 'Too many sync wait commands'). Move the
    excess onto same-engine NoOps inserted just before the instruction —
    the engine blocks on the NoOps' waits first, so semantics match."""
    idx = 0
    for f in nc.m.functions:
        for bb in f.blocks:
            new = []
            changed = False
            for inst in bb.instructions:
                si = getattr(inst, "sync_info", None)
                waits = list(si.on_wait) if si is not None and si.on_wait else []
                if (
                    len(waits) > _MAX_WAITS
                    and inst.engine != mybir.EngineType.Unassigned
                ):
                    changed = True
                    excess = waits[: -_MAX_WAITS]
                    for j in range(0, len(excess), _MAX_WAITS):
                        nop = mybir.InstNoOp(
                            name=f"wspill-{idx}",
                            bass_nofuse=True,
                            sync_info=mybir.SyncInfo(
                                on_wait=excess[j : j + _MAX_WAITS], on_update=[]
                            ),
                        )
                        idx += 1
                        nop.engine = inst.engine
                        nc.register_instruction(nop)
                        new.append(nop)
                    si.on_wait = waits[-_MAX_WAITS:]
                new.append(inst)
            if changed:
                bb.instructions = new


_orig_drain_and_barrier = tile.TileContext._drain_and_barrier


def _drain_barrier_and_spill(self, tick_clock, wait_clock):
    _orig_drain_and_barrier(self, tick_clock, wait_clock)
    _spill_excess_waits(self.nc)


tile.TileContext._drain_and_barrier = _drain_barrier_and_spill

P = 128
S = 2048
D = 1024
DH = 64
NH_TOT = 16
NHC = 4  # heads per core
NG = 2  # score groups
LAMBDA_INIT = 0.8
NCORES = 8

F32 = mybir.dt.float32
F32R = mybir.dt.float32r
BF16 = mybir.dt.bfloat16
F16 = mybir.dt.float16
EXP = mybir.ActivationFunctionType.Exp
MULT = mybir.AluOpType.mult
IS_GE = mybir.AluOpType.is_ge

TOKC = S // P  # 16 token chunks
DC = D // P  # 8 d_model chunks
QB = 512  # q block width for score matmuls
NQB = S // QB  # 4
WCOLS = NHC * NG * DH  # 512 projection cols per core
OROWS = NHC * DH  # 256 o_proj rows per core

LAST_RESULT = None  # test harness reads exec_time_ns from here


def _r(ap):
    return ap.bitcast(F32R)


def build_program(c1: float, c2: float) -> bass.Bass:
    """c1 = (1-lambda_init), c2 = (1-lambda_init)*lambda — baked immediates."""
    nc = bass.Bass("TRN2", target_bir_lowering=False, debug=False)

    xb = nc.dram_tensor("xb", [S, D], F32, kind="ExternalInput").ap()
    wq = nc.dram_tensor("wq", [D, WCOLS], F32, kind="ExternalInput").ap()
    wk = nc.dram_tensor("wk", [D, WCOLS], F32, kind="ExternalInput").ap()
    wv = nc.dram_tensor("wv", [D, WCOLS], F32, kind="ExternalInput").ap()
    wo = nc.dram_tensor("wo", [OROWS, D], F32, kind="ExternalInput").ap()
    out = nc.dram_tensor("out", [S, D], F16, kind="ExternalOutput").ap()

    NQ = 4  # token quarters (== q blocks)

    with tile.TileContext(nc) as tc, ExitStack() as es:
        pool = es.enter_context(tc.tile_pool(name="main", bufs=1))
        ident16 = pool.tile([P, P], F16)
        make_identity(nc, ident16)

        # Tensors are split per token quarter so the Tile scheduler can
        # overlap attention on early quarters with projections of later ones.
        qTq = [pool.tile([P, NHC, QB], F16, name=f"qT{j}") for j in range(NQ)]
        # k^T strips: per (head, group), the group's 64 rows at their natural
        # position, zeros in the other half — a full-128 stationary masks the
        # other group in the stacked q^T moving operand (K=128 keeps FWL on;
        # K=64 row-tiled pairs measured slower)
        kTq = [pool.tile([P, NHC * NG, QB], F16, name=f"kT{j}") for j in range(NQ)]
        vSq = [
            pool.tile([P, 4, NHC * NG, DH + 1], F16, name=f"vS{j}") for j in range(NQ)
        ]
        xTq = [pool.tile([P, DC, QB], F16, name=f"xT{j}") for j in range(NQ)]

        xs_pool = es.enter_context(tc.tile_pool(name="xs", bufs=5))
        xc_pool = es.enter_context(tc.tile_pool(name="xc", bufs=4))
        a_pool = es.enter_context(tc.tile_pool(name="a", bufs=6))
        nrm_pool = es.enter_context(tc.tile_pool(name="nrm", bufs=4))
        od_pool = es.enter_context(tc.tile_pool(name="odq", bufs=2))
        odT_pool = es.enter_context(tc.tile_pool(name="odT", bufs=2))
        outs_pool = es.enter_context(tc.tile_pool(name="outs", bufs=4))
        # PSUM (8 banks): 2 proj/o_proj/transpose (tag ps), 2x2-bank score
        # tiles (both groups side by side), 2 AV accumulators
        pp512 = es.enter_context(tc.tile_pool(name="pp512", bufs=2, space="PSUM"))
        s_psum = es.enter_context(tc.tile_pool(name="sps", bufs=2, space="PSUM"))
        o_psum = es.enter_context(tc.tile_pool(name="ops", bufs=2, space="PSUM"))

        # HAM warm-up: ~36 back-to-back dummy matmuls keep the PE busy for
        # >3.4us starting at t~8.5us, flipping the clock gate to 8/8 before
        # the real transposes/projections arrive (they'd otherwise run at
        # 1.2GHz until ~40us in).
        warm = pp512.tile([P, P], F32, tag="ps", name="warm")
        for _ in range(24):
            nc.tensor.matmul(
                warm[:], lhsT=ident16[:], rhs=ident16[:], start=True, stop=True
            )

        # ---- projections, one token quarter at a time ----
        for j in range(NQ):
            for ti in range(4):
                t = j * 4 + ti
                xstage = xs_pool.tile([P, D], F32, tag="xs", name="xs")
                nc.sync.dma_start(xstage[:], xb[t * P : (t + 1) * P, :])
                xc = xc_pool.tile([P, D], F16, tag="xc", name="xc")
                if ti % 2:
                    nc.scalar.copy(xc[:], xstage[:])
                else:
                    nc.vector.tensor_copy(xc[:], xstage[:])
                for dq in range(DC // 4):
                    # transpose as regular matmuls (stationary = chunk,
                    # moving = identity): ~2x faster than transpose-mode.
                    # 4 transposes land in one psum bank (disjoint regions,
                    # start only on the first clears has_written) so a single
                    # wide copy drains them.
                    pt4 = o_psum.tile([P, 4, P], F32, tag="og", name="tp")
                    for c in range(4):
                        dc = dq * 4 + c
                        nc.tensor.matmul(
                            pt4[:, c, :],
                            lhsT=xc[:, dc * P : (dc + 1) * P],
                            rhs=ident16[:],
                            start=(c == 0),
                            stop=(c == 3),
                        )
                    nc.vector.tensor_copy(
                        xTq[j][:, dq * 4 : dq * 4 + 4, ti * P : (ti + 1) * P],
                        pt4[:],
                    )
            if j == 0:
                # weights + constants load after quarter-0's x pipeline is
                # queued, so the PE can start transposing immediately
                wst_pool = es.enter_context(tc.tile_pool(name="wst", bufs=5))
                w16 = {}
                nw = 0
                for nm, wdram in (("q", wq), ("k", wk), ("v", wv)):
                    w16[nm] = [
                        pool.tile([P, WCOLS], F16, name=f"w16{nm}{dc}") for dc in range(DC)
                    ]
                    for dc in range(DC):
                        wst = wst_pool.tile([P, WCOLS], F32, tag="wst", name="wst")
                        nc.sync.dma_start(wst[:], wdram[dc * P : (dc + 1) * P, :])
                        # alternate engines so neither serializes the PE feed
                        eng = nc.scalar.copy if nw % 2 else nc.vector.tensor_copy
                        eng(w16[nm][dc][:], wst[:])
                        nw += 1
                wos = pool.tile([P, OROWS // P, D], F16)
            if j == 1:
                # wo is not needed until the first o_proj (~t=75us); load it
                # after quarter-1's x so it delays neither
                for mc in range(OROWS // P):
                    wst = wst_pool.tile([P, D], F32, tag="wst", name="wost")
                    nc.sync.dma_start(wst[:], wo[mc * P : (mc + 1) * P, :])
                    nc.scalar.copy(wos[:, mc, :], wst[:])
                for jj in range(NQ):
                    for g in range(NG):
                        other = (1 - g) * DH
                        for hh in range(NHC):
                            nc.gpsimd.memset(
                                kTq[jj][other : other + DH, 2 * hh + g, :], 0.0
                            )
                    nc.gpsimd.memset(vSq[jj][:, :, :, DH], 1.0)

            # q^T, k^T: out[dims 128, tok 512]; one live psum per dim chunk.
            # PSUM->SBUF copies go on the Scalar engine, idle in this phase
            # (the DVE is busy with x casts + transpose copies).
            for nm in ("q", "k"):
                for mc in range(NHC):
                    ps = pp512.tile([P, QB], F32, tag="ps", name="ps")
                    for dc in range(DC):
                        nc.tensor.matmul(
                            ps[:],
                            lhsT=w16[nm][dc][:, mc * P : (mc + 1) * P],
                            rhs=xTq[j][:, dc, :],
                            start=(dc == 0),
                            stop=(dc == DC - 1),
                        )
                    if nm == "q":
                        eng = nc.scalar.copy if mc % 2 else nc.vector.tensor_copy
                        eng(qTq[j][:, mc, :], ps[:])
                    else:
                        eng = nc.scalar.copy if mc % 2 else nc.vector.tensor_copy
                        for g in range(NG):
                            eng(
                                kTq[j][g * DH : (g + 1) * DH, 2 * mc + g, :],
                                ps[g * DH : (g + 1) * DH, :],
                            )
            # v: out[tok 128, strips 512]
            for ti in range(4):
                ps = pp512.tile([P, QB], F32, tag="ps", name="ps")
                for dc in range(DC):
                    nc.tensor.matmul(
                        ps[:],
                        lhsT=xTq[j][:, dc, ti * P : (ti + 1) * P],
                        rhs=w16["v"][dc][:],
                        start=(dc == 0),
                        stop=(dc == DC - 1),
                    )
                eng = nc.scalar.copy if ti % 2 else nc.vector.tensor_copy
                eng(
                    vSq[j][:, ti, :, 0:DH],
                    ps[:].rearrange("p (s d) -> p s d", s=NHC * NG),
                )

        # ---- attention + per-q-block o_proj ----
        for qb in range(NQB):
            o_dq = od_pool.tile([P, 4, OROWS], F16, tag="odq", name="odq")

            def do_av(hh, og, kc, at):
                kj, ki = kc // 4, kc % 4
                for g in range(NG):
                    strip = 2 * hh + g
                    for qs in range(4):
                        if kc - 4 * qb > qs:
                            continue  # fully masked sub-block
                        # one accumulation group per og bank: the first
                        # matmul's start clears has_written for the whole
                        # bank; later matmuls overwrite where unwritten /
                        # accumulate where written
                        nc.tensor.matmul(
                            og[g][:, qs, :],
                            lhsT=at[:, g, qs * P : (qs + 1) * P],
                            rhs=vSq[kj][:, ki, strip, :],
                            start=(kc == 0 and qs == 0),
                            stop=(kc == 4 * qb + 3 and qs == 3),
                        )

            def emit_norm(hh, og):
                # normalize rows, combine groups: o = c1*o1/s1 - c2*o2/s2
                rc = [
                    nrm_pool.tile([P, 4, 1], F32, tag="rc", name="rc")
                    for _ in range(NG)
                ]
                for g in range(NG):
                    nc.vector.reciprocal(rc[g][:], og[g][:, :, DH : DH + 1])
                    nc.vector.tensor_scalar_mul(
                        rc[g][:], rc[g][:], c1 if g == 0 else -c2
                    )
                t0 = nrm_pool.tile([P, 4, DH], F32, tag="tt")
                t1 = nrm_pool.tile([P, 4, DH], F32, tag="tt")
                nc.vector.tensor_tensor(
                    t0[:], og[0][:, :, 0:DH], rc[0][:].to_broadcast([P, 4, DH]), MULT
                )
                nc.vector.tensor_tensor(
                    t1[:], og[1][:, :, 0:DH], rc[1][:].to_broadcast([P, 4, DH]), MULT
                )
                nc.vector.tensor_add(
                    o_dq[:, :, hh * DH : (hh + 1) * DH], t0[:], t1[:]
                )

            # software pipeline ACROSS head chains: AV of chunk kc is emitted
            # two score/exp groups later (possibly inside the next head's
            # chain), so AV stationary loads prefetch under long score
            # matmuls and a chain's final exp lag is absorbed by the next
            # chain's scores instead of stalling the PE FIFO
            kmax = 4 * (qb + 1) - 1
            pendq = []

            def pop_av():
                hh_, og_, kc_, at_ = pendq.pop(0)
                do_av(hh_, og_, kc_, at_)
                if kc_ == kmax:
                    emit_norm(hh_, og_)

            for hh in range(NHC):
                og = [
                    o_psum.tile([P, 4, DH + 1], F32, tag="og", name="og")
                    for _ in range(NG)
                ]
                for kc in range(4 * (qb + 1)):
                    kj, ki = kc // 4, kc % 4
                    r = max(0, (kc - 4 * qb) * P)
                    sp = s_psum.tile([P, NG, QB], F32, tag="sp", name="sp")
                    at = a_pool.tile([P, NG, QB], F16, tag="at", name="at")
                    # stacked-group trick: the zero-padded kT strip masks the
                    # other group, so K=128 (full FWL rate) per group
                    for g in range(NG):
                        nc.tensor.matmul(
                            sp[:, g, r:QB],
                            lhsT=kTq[kj][:, 2 * hh + g, ki * P : (ki + 1) * P],
                            rhs=qTq[qb][:, hh, r:QB],
                            start=True,
                            stop=True,
                        )
                    # one EXP for both groups (no max-subtraction: |s|<3)
                    nc.scalar.activation(
                        at[:, :, r:QB], sp[:, :, r:QB], EXP, scale=0.125
                    )
                    if kc >= 4 * qb:
                        # band [r, r+128): keep where col >= row
                        for g in range(NG):
                            nc.gpsimd.affine_select(
                                out=at[:, g, r : r + P],
                                in_=at[:, g, r : r + P],
                                compare_op=IS_GE,
                                fill=0.0,
                                base=0,
                                pattern=[[1, P]],
                                channel_multiplier=-1,
                            )
                    if len(pendq) >= 2:
                        pop_av()
                    pendq.append((hh, og, kc, at))
            while pendq:
                pop_av()
            # o_proj for this q block, hidden under later attention
            odT = odT_pool.tile([P, OROWS // P, 4 * P], F16, tag="odT", name="odT")
            for mc in range(OROWS // P):
                pt4 = o_psum.tile([P, 4, P], F32, tag="og", name="tp2")
                for tix in range(4):
                    nc.tensor.matmul(
                        pt4[:, tix, :],
                        lhsT=o_dq[:, tix, mc * P : (mc + 1) * P],
                        rhs=ident16[:],
                        start=(tix == 0),
                        stop=(tix == 3),
                    )
                nc.vector.tensor_copy(
                    odT[:, mc, :],
                    pt4[:].rearrange("p c w -> p (c w)"),
                )
            for tix in range(4):
                t = qb * 4 + tix
                for nb in range(D // QB):
                    op = pp512.tile([P, QB], F32, tag="ps", name="op")
                    for mc in range(OROWS // P):
                        nc.tensor.matmul(
                            op[:],
                            lhsT=odT[:, mc, tix * P : (tix + 1) * P],
                            rhs=wos[:, mc, nb * QB : (nb + 1) * QB],
                            start=(mc == 0),
                            stop=(mc == OROWS // P - 1),
                        )
                    ot = outs_pool.tile([P, QB], F16, tag="ot", name="ot")
                    nc.vector.tensor_copy(ot[:], op[:])
                    nc.sync.dma_start(
                        out[t * P : (t + 1) * P, nb * QB : (nb + 1) * QB], ot[:]
                    )

    return nc


_PROGRAM_CACHE: dict = {}


def _get_program(c1: float, c2: float) -> bass.Bass:
    key = (round(c1, 12), round(c2, 12))
    if key not in _PROGRAM_CACHE:
        _PROGRAM_CACHE[key] = build_program(c1, c2)
    return _PROGRAM_CACHE[key]


def make_in_maps(x, Wq, Wk, Wv, Wo):
    """Shard full inputs into the 8 per-core input dicts."""
    x = np.asarray(x, np.float32)
    in_maps = []
    for c in range(NCORES):
        b, qd = divmod(c, 4)
        cols = np.concatenate(
            [
                np.arange(DH) + g * (NH_TOT * DH) + (4 * qd + hh) * DH
                for hh in range(NHC)
                for g in range(NG)
            ]
        )
        in_maps.append(
            {
                "xb": np.ascontiguousarray(x[b]),
                "wq": np.ascontiguousarray(np.asarray(Wq, np.float32)[:, cols]),
                "wk": np.ascontiguousarray(np.asarray(Wk, np.float32)[:, cols]),
                "wv": np.ascontiguousarray(np.asarray(Wv, np.float32)[:, cols]),
                "wo": np.ascontiguousarray(
                    np.asarray(Wo, np.float32)[qd * OROWS : (qd + 1) * OROWS, :]
                ),
            }
        )
    return in_maps


def kernel(x, Wq, Wk, Wv, Wo, lq1, lk1, lq2, lk2):
    global LAST_RESULT
    lam = float(
        np.exp(np.float32(np.dot(lq1, lk1)))
        - np.exp(np.float32(np.dot(lq2, lk2)))
        + np.float32(LAMBDA_INIT)
    )
    c1 = 1.0 - LAMBDA_INIT
    c2 = (1.0 - LAMBDA_INIT) * lam
    nc = _get_program(c1, c2)
    in_maps = make_in_maps(x, Wq, Wk, Wv, Wo)
    res = run_bass_kernel_spmd(nc, in_maps, list(range(NCORES)))
    LAST_RESULT = res
    B = 2
    out64 = np.zeros((B, S, D), np.float64)
    for c in range(NCORES):
        out64[c // 4] += res.results[c]["out"].astype(np.float64)
    return out64.astype(np.float32)
